# revision 1
# baseline (speedup 1.0000x reference)
"""Trainium2 Bass kernel (raw Bass, no Tile): per-class precision/recall sums.

Computes, for pred/gt 0-1 indicator tensors of shape [N, C]:
    intersection = sum_n pred*gt   [C]
    pred_sum     = sum_n pred      [C]
    gt_sum       = sum_n gt        [C]
    precisions   = (intersection + EPS) / (pred_sum + EPS)
    recalls      = (intersection + EPS) / (gt_sum + EPS)

Sharding: rows split across 8 NeuronCores. The host interleaves each
core's pred/gt chunks in 8-row blocks into x[R/8, 2, 8, C] so one DMA
per tile feeds both operands (each 128-element run purely pred or gt).
Each core emits a [1, 3*C] partial; the host sums partials (integer
values < 2^24, exact in fp32) and applies the epsilon math.

Device pipeline per core (memory-bound, 64 MiB HBM traffic):
  - gpsimd SWDGE DMAs cast f32 -> bf16 in flight (exact for 0/1):
    32 tiles xt[128, 4096] bf16 into 16 rotating SBUF slots.
  - TensorE does all the math:
    * ones[128,1]^T @ 512-col slices accumulate ps_sums[1,512].
    * Gram matmuls pred_run^T @ gt_run per 256-col block accumulate
      ps_gram[128,128]; diagonal entry a=(r,c) = pred.gt dot.
  - Epilogue: diag-mask ps_gram (affine_select identity), fp32
    ones-matmul column-sum -> ps_row[1,128], strided reduces fold into
    res[1,48] = [pred_sum, gt_sum, intersection].

Raw Bass because this compiler build encodes at most ONE semaphore wait
per TPB instruction: all multi-condition waits are standalone sequencer
wait_ge instructions. Correctness argument for slot recycling: the DMA
for tile t (t>=16) waits pe_sem >= t-15, i.e. PE finished reading tile
t-16 in that slot, which also implies that DMA t-16 completed.
Per-slot DMA-completion sems make PE's data waits exact even if the
runtime spreads DMAs across queues that complete out of order.
"""

from contextlib import ExitStack

import numpy as np

N_CORES = 8
N_ROWS, C = 4194304, 16
ROWS_PER_CORE = N_ROWS // N_CORES  # 524288
EPS = np.float32(1e-6)

P = 128
ELEMS_PER_CORE = ROWS_PER_CORE * 2 * C      # 16777216
FREE = 8192          # bf16 elements per partition per tile
TILE_ELEMS = P * FREE                       # 1048576
N_TILES = ELEMS_PER_CORE // TILE_ELEMS      # 16
N_SLOTS = 10
MM_FREE = 512
N_SUM_SLICES = FREE // MM_FREE              # 16
GRAM_BLK = 256       # (two=2, r=8, c=16)
N_GRAM_BLKS = FREE // GRAM_BLK              # 32

_CACHE = {}
LAST_RUN = None  # BassKernelResults of the most recent run (for test harness)


def _build_nc():
    import concourse.bass as bass
    import concourse.mybir as mybir

    f32 = mybir.dt.float32
    bf16 = mybir.dt.bfloat16

    nc = bass.Bass()
    x_d = nc.dram_tensor("x", [ROWS_PER_CORE // 8, 2, 8, C], f32,
                         kind="ExternalInput")
    out_d = nc.dram_tensor("out", [1, 3 * C], f32, kind="ExternalOutput")

    x_t = x_d[:, :, :, :].rearrange("(t p f) two r c -> t p (f two r c)",
                                    p=P, f=FREE // GRAM_BLK)

    ctx = ExitStack()
    with ctx:
        ones_b = ctx.enter_context(nc.sbuf_tensor("ones_b", [P, 1], bf16))
        ones_f = ctx.enter_context(nc.sbuf_tensor("ones_f", [P, 1], f32))
        onesI = ctx.enter_context(nc.sbuf_tensor("onesI", [P, P], f32))
        ident = ctx.enter_context(nc.sbuf_tensor("ident", [P, P], f32))
        diag = ctx.enter_context(nc.sbuf_tensor("diag", [P, P], f32))
        sum4 = ctx.enter_context(nc.sbuf_tensor("sum4", [1, 4 * C], f32))
        res = ctx.enter_context(nc.sbuf_tensor("res", [1, 3 * C], f32))
        slots = [
            ctx.enter_context(nc.sbuf_tensor(f"xt{s}", [P, FREE], bf16))
            for s in range(N_SLOTS)
        ]

        ps_sums = ctx.enter_context(nc.psum_tensor([1, MM_FREE], f32))
        ps_gram = ctx.enter_context(nc.psum_tensor([P, P], f32))
        ps_row = ctx.enter_context(nc.psum_tensor([1, P], f32))

        slot_sems = [
            ctx.enter_context(nc.semaphore(name=f"slot{s}"))
            for s in range(N_SLOTS)
        ]
        qsems = [
            ctx.enter_context(nc.semaphore(name=f"q{k}"))
            for k in range(4)
        ]
        pe_sem = ctx.enter_context(nc.semaphore(name="pe"))
        dve_sem = ctx.enter_context(nc.semaphore(name="dve"))
        pool_sem = ctx.enter_context(nc.semaphore(name="pool"))
        out_sem = ctx.enter_context(nc.semaphore(name="outd"))
        block = ctx.enter_context(nc.Block())

        @block.gpsimd
        def _(gpsimd):
            gpsimd.memset(onesI[:], 1.0)
            gpsimd.affine_select(ident[:], onesI[:], [[1, P]],
                                 mybir.AluOpType.is_equal, 0.0,
                                 base=0, channel_multiplier=-1)
            gpsimd.nop().then_inc(pool_sem, 1)
            for t in range(N_TILES):
                s = t % N_SLOTS
                if t >= N_SLOTS:
                    # PE finished reading the previous occupant of this slot
                    gpsimd.wait_ge(pe_sem, t - N_SLOTS + 1)
                if t < N_TILES - 1:
                    gpsimd.dma_start(slots[s][:], x_t[t]).then_inc(
                        slot_sems[s], 16)
                else:
                    # last tile: 4 quarter-DMAs so PE can chase the stream
                    # and finish right after the final byte lands
                    q = FREE // 4
                    for k in range(4):
                        gpsimd.dma_start(
                            slots[s][:, k * q:(k + 1) * q],
                            x_t[t][:, k * q:(k + 1) * q],
                        ).then_inc(qsems[k], 16)
            # final output DMA after DVE finishes the epilogue
            gpsimd.wait_ge(dve_sem, 3)
            gpsimd.dma_start(out_d[:, :], res[:]).then_inc(out_sem, 16)
            gpsimd.wait_ge(out_sem, 16)

        @block.vector
        def _(vector):
            vector.memset(ones_b[:], 1.0)
            vector.memset(ones_f[:], 1.0)
            vector.nop().then_inc(dve_sem, 1)
            # epilogue part 1: after all accumulation matmuls
            vector.wait_ge(pe_sem, N_TILES)
            vector.wait_ge(pool_sem, 1)
            vector.tensor_mul(diag[:], ps_gram[:, :], ident[:])
            vector.tensor_reduce(
                sum4[:],
                ps_sums[:, :].rearrange("p (b2 two r c) -> p b2 two c r",
                                        b2=2, two=2, r=8, c=C),
                axis=mybir.AxisListType.X, op=mybir.AluOpType.add)
            vector.tensor_reduce(
                res[0:1, 0:2 * C],
                sum4[:, :].rearrange("p (b2 tc) -> p tc b2", b2=2, tc=2 * C),
                axis=mybir.AxisListType.X, op=mybir.AluOpType.add)
            vector.nop().then_inc(dve_sem, 1)  # diag + sums folded
            # epilogue part 2: after PE's diag column-sum matmul
            vector.wait_ge(pe_sem, N_TILES + 1)
            vector.tensor_reduce(
                res[0:1, 2 * C:3 * C],
                ps_row[:, :].rearrange("p (g c) -> p c g", g=8, c=C),
                axis=mybir.AxisListType.X, op=mybir.AluOpType.add)
            vector.nop().then_inc(dve_sem, 1)

        @block.tensor
        def _(tensor):
            tensor.wait_ge(dve_sem, 1)  # ones_b / ones_f ready
            for t in range(N_TILES):
                s = t % N_SLOTS
                xt = slots[s]
                quarters = 1 if t < N_TILES - 1 else 4
                if quarters == 1:
                    tensor.wait_ge(slot_sems[s], 16 * (t // N_SLOTS + 1))
                for k in range(quarters):
                    if quarters == 4:
                        tensor.wait_ge(qsems[k], 16)
                    nsum = N_SUM_SLICES // quarters
                    ngram = N_GRAM_BLKS // quarters
                    for i in range(k * nsum, (k + 1) * nsum):
                        mm = t * N_SUM_SLICES + i
                        nc.tensor.matmul(
                            ps_sums[:, :], ones_b[:],
                            xt[:, i * MM_FREE:(i + 1) * MM_FREE],
                            start=(mm == 0),
                            stop=(mm == N_TILES * N_SUM_SLICES - 1))
                    for j in range(k * ngram, (k + 1) * ngram):
                        mm = t * N_GRAM_BLKS + j
                        base = j * GRAM_BLK
                        mminst = nc.tensor.matmul(
                            ps_gram[:, :], xt[:, base:base + P],
                            xt[:, base + P:base + 2 * P],
                            start=(mm == 0),
                            stop=(mm == N_TILES * N_GRAM_BLKS - 1))
                        if j == N_GRAM_BLKS - 1:
                            mminst.then_inc(pe_sem, 1)
            # epilogue: fp32 column-sum of masked diagonal
            tensor.wait_ge(dve_sem, 2)
            nc.tensor.matmul(ps_row[:, :], ones_f[:], diag[:],
                             start=True, stop=True).then_inc(pe_sem, 1)

    return nc


def _get_nc():
    if "nc" not in _CACHE:
        _CACHE["nc"] = _build_nc()
    return _CACHE["nc"]


def kernel(pred, gt, **run_kwargs):
    global LAST_RUN
    from concourse.bass_utils import run_bass_kernel_spmd

    pred = np.asarray(pred, dtype=np.float32)
    gt = np.asarray(gt, dtype=np.float32)
    assert pred.shape == (N_ROWS, C) and gt.shape == (N_ROWS, C)

    in_maps = []
    for i in range(N_CORES):
        sl = slice(i * ROWS_PER_CORE, (i + 1) * ROWS_PER_CORE)
        x = np.empty((ROWS_PER_CORE // 8, 2, 8, C), dtype=np.float32)
        x[:, 0, :, :] = pred[sl].reshape(-1, 8, C)
        x[:, 1, :, :] = gt[sl].reshape(-1, 8, C)
        in_maps.append({"x": x})

    nc = _get_nc()
    br = run_bass_kernel_spmd(nc, in_maps, core_ids=list(range(N_CORES)),
                              **run_kwargs)
    LAST_RUN = br

    partials = np.stack([r["out"].reshape(3 * C) for r in br.results])
    totals = partials.astype(np.float64).sum(axis=0)  # exact integers
    pred_sum = totals[0:C].astype(np.float32)
    gt_sum = totals[C:2 * C].astype(np.float32)
    intersection = totals[2 * C:3 * C].astype(np.float32)

    recalls = (intersection + EPS) / (gt_sum + EPS)
    precisions = (intersection + EPS) / (pred_sum + EPS)
    return (precisions, recalls, intersection, gt_sum, pred_sum)



# revision 11
# speedup vs baseline: 1.7209x; 1.7209x over previous
"""Trainium2 Bass kernel (raw Bass): per-class precision/recall sums.

Computes, for pred/gt 0-1 indicator tensors of shape [N, C]:
    intersection = sum_n pred*gt   [C]
    pred_sum     = sum_n pred      [C]
    gt_sum       = sum_n gt        [C]
    precisions   = (intersection + EPS) / (pred_sum + EPS)
    recalls      = (intersection + EPS) / (gt_sum + EPS)

Sharding: rows split across 8 NeuronCores. The host packs each core's
chunk as bf16 (exact for 0/1 indicators - truncating the f32 top half)
into x[16, 128, 8192]: tile t, partition p holds 256 consecutive rows;
free layout = [pred (q256 c16) | gt (q256 c16)]. bf16 on the wire halves
HBM traffic vs f32: 32 MiB/core, ~79 us at the 16x27GB/s DMA-engine
roofline.

Device pipeline per core:
  - gpsimd SWDGE streams 16 tiles xt[128, 8192] bf16 into 8 rotating
    SBUF slots (no dtype cast - host already packed bf16). Last tile
    split into 4 quarter-DMAs so compute can chase the stream.
  - DVE: z = pred_half * gt_half per tile (one [128,4096] bf16 mul).
  - PE: ones[128,1]^T @ 512-col slices; pred slices accumulate
    psA[1,512], gt slices psB[1,512], z slices psC[1,512]. Stationary
    ones never changes; 24 matmuls/tile, ~420 PE instructions total
    (small enough to avoid mid-stream iram refills, which made DMA
    engine 64 a straggler in the f32/gram version).
  - Epilogue: DVE strided reduces psA/psB/psC -> res[1,48]; sync-engine
    HWDGE writes res to HBM as a single descriptor.
Each core emits [1, 3*C] = [pred_sum, gt_sum, intersection]; the host
sums partials (exact integers in f64) and applies the epsilon math.
"""

from contextlib import ExitStack

import numpy as np

N_CORES = 8
N_ROWS, C = 4194304, 16
ROWS_PER_CORE = N_ROWS // N_CORES  # 524288
EPS = np.float32(1e-6)

P = 128
N_TILES = 16
Q = ROWS_PER_CORE // (N_TILES * P)  # 256 rows per (tile, partition)
HALF = Q * C                        # 4096
FREE = 2 * HALF                     # 8192
N_SLOTS = 8
NZ = 3
MM = 512                            # moving free per matmul
NSL = HALF // MM                    # 8 slices per half
NZSL = HALF // MM                   # 8 slices of z

_CACHE = {}
LAST_RUN = None  # BassKernelResults of the most recent run (for test harness)
DEBUG_DUMP = False  # add raw psum + z-tile debug outputs


def _build_nc():
    import concourse.bass as bass
    import concourse.mybir as mybir

    f32 = mybir.dt.float32
    bf16 = mybir.dt.bfloat16

    nc = bass.Bass()
    x_d = nc.dram_tensor("x", [N_TILES, P, FREE], bf16, kind="ExternalInput")
    out_d = nc.dram_tensor("out", [1, 3 * C], f32, kind="ExternalOutput")
    if DEBUG_DUMP:
        ps_d = nc.dram_tensor("psdump", [1, 3 * MM], f32, kind="ExternalOutput")
        z_d = nc.dram_tensor("zdump", [P, HALF], bf16, kind="ExternalOutput")
    x_t = x_d[:, :, :]

    ctx = ExitStack()
    with ctx:
        ones_b = ctx.enter_context(nc.sbuf_tensor("ones_b", [P, 1], bf16))
        res = ctx.enter_context(nc.sbuf_tensor("res", [1, 3 * C], f32))
        res2 = ctx.enter_context(nc.sbuf_tensor("res2", [1, 3 * C], f32))
        if DEBUG_DUMP:
            psraw = ctx.enter_context(nc.sbuf_tensor("psraw", [1, 3 * MM], f32))
        slots = [
            ctx.enter_context(nc.sbuf_tensor(f"xt{s}", [P, FREE], bf16))
            for s in range(N_SLOTS)
        ]
        zslots = [
            ctx.enter_context(nc.sbuf_tensor(f"z{s}", [P, HALF], bf16))
            for s in range(NZ)
        ]

        psA = ctx.enter_context(nc.psum_tensor([1, MM], f32))
        psB = ctx.enter_context(nc.psum_tensor([1, MM], f32))
        psC = ctx.enter_context(nc.psum_tensor([1, MM], f32))

        slot_sems = [
            ctx.enter_context(nc.semaphore(name=f"slot{s}"))
            for s in range(N_SLOTS)
        ]
        qsems = [
            ctx.enter_context(nc.semaphore(name=f"q{k}"))
            for k in range(4)
        ]
        z_sem = ctx.enter_context(nc.semaphore(name="zs"))
        pe_sem = ctx.enter_context(nc.semaphore(name="pe"))
        dve_sem = ctx.enter_context(nc.semaphore(name="dve"))
        out_sem = ctx.enter_context(nc.semaphore(name="outd"))
        block = ctx.enter_context(nc.Block())

        # last tile quarter ranges, issue order: pred_h0, gt_h0, pred_h1,
        # gt_h1 so DVE can mul half 0 while halves 1 stream in.
        QTR = [(0, HALF // 2), (HALF, HALF + HALF // 2),
               (HALF // 2, HALF), (HALF + HALF // 2, FREE)]

        @block.gpsimd
        def _(gpsimd):
            for t in range(N_TILES):
                s = t % N_SLOTS
                if t >= N_SLOTS:
                    # PE finished all matmuls of the slot's previous tile,
                    # which also implies DVE's mul of it completed.
                    gpsimd.wait_ge(pe_sem, t - N_SLOTS + 1)
                if t < N_TILES - 1:
                    gpsimd.dma_start(slots[s][:], x_t[t]).then_inc(
                        slot_sems[s], 16)
                else:
                    for k, (lo, hi) in enumerate(QTR):
                        gpsimd.dma_start(
                            slots[s][:, lo:hi], x_t[t][:, lo:hi],
                        ).then_inc(qsems[k], 16)

        @block.vector
        def _(vector):
            # inc must ride ON the writing instruction: a trailing nop's
            # inc can fire while the previous op's writes are in flight.
            vector.memset(ones_b[:], 1.0).then_inc(dve_sem, 1)
            for t in range(N_TILES - 1):
                s = t % N_SLOTS
                if t >= NZ:
                    vector.wait_ge(pe_sem, t - NZ + 1)
                vector.wait_ge(slot_sems[s], 16 * (t // N_SLOTS + 1))
                vector.tensor_mul(
                    zslots[t % NZ][:],
                    slots[s][:, 0:HALF],
                    slots[s][:, HALF:FREE],
                ).then_inc(z_sem, 1)
            # last tile: two half-muls chasing the quarter DMAs
            t = N_TILES - 1
            s = t % N_SLOTS
            zz = zslots[t % NZ]
            vector.wait_ge(pe_sem, t - NZ + 1)
            h = HALF // 2
            vector.wait_ge(qsems[1], 16)
            vector.tensor_mul(
                zz[:, 0:h], slots[s][:, 0:h], slots[s][:, HALF:HALF + h],
            ).then_inc(z_sem, 1)
            vector.wait_ge(qsems[3], 16)
            vector.tensor_mul(
                zz[:, h:HALF], slots[s][:, h:HALF], slots[s][:, HALF + h:FREE],
            ).then_inc(z_sem, 1)
            # epilogue: decode psum accumulators once PE retires
            vector.wait_ge(pe_sem, N_TILES)
            vector.tensor_reduce(
                res[0:1, 0:C],
                psA[:, :].rearrange("p (q c) -> p c q", c=C),
                axis=mybir.AxisListType.X, op=mybir.AluOpType.add)
            vector.tensor_reduce(
                res[0:1, C:2 * C],
                psB[:, :].rearrange("p (q c) -> p c q", c=C),
                axis=mybir.AxisListType.X, op=mybir.AluOpType.add)
            vector.tensor_reduce(
                res[0:1, 2 * C:3 * C],
                psC[:, :].rearrange("p (q c) -> p c q", c=C),
                axis=mybir.AxisListType.X, op=mybir.AluOpType.add)
            if DEBUG_DUMP:
                vector.tensor_copy(psraw[0:1, 0:MM], psA[:, :])
                vector.tensor_copy(psraw[0:1, MM:2 * MM], psB[:, :])
                vector.tensor_copy(psraw[0:1, 2 * MM:3 * MM], psC[:, :])
            # read-back layer: the copy can't read res before the reduces'
            # writes land (in-order same-engine RAW), and its own inc rides
            # on the instruction that writes res2 - the DMA reads res2.
            vector.tensor_copy(res2[0:1, :], res[0:1, :]).then_inc(dve_sem, 1)

        @block.tensor
        def _(tensor):
            tensor.wait_ge(dve_sem, 1)  # ones_b ready
            n_mm = N_TILES * NSL
            n_zmm = N_TILES * NZSL
            for t in range(N_TILES):
                s = t % N_SLOTS
                xt = slots[s]
                zz = zslots[t % NZ]
                if t < N_TILES - 1:
                    tensor.wait_ge(slot_sems[s], 16 * (t // N_SLOTS + 1))
                    for i in range(NSL):
                        nc.tensor.matmul(
                            psA[:, :], ones_b[:],
                            xt[:, i * MM:(i + 1) * MM],
                            start=(t == 0 and i == 0),
                            stop=(t == N_TILES - 1 and i == NSL - 1))
                    for i in range(NSL):
                        nc.tensor.matmul(
                            psB[:, :], ones_b[:],
                            xt[:, HALF + i * MM:HALF + (i + 1) * MM],
                            start=(t == 0 and i == 0),
                            stop=(t == N_TILES - 1 and i == NSL - 1))
                    tensor.wait_ge(z_sem, t + 1)
                    for j in range(NZSL):
                        mm = nc.tensor.matmul(
                            psC[:, :], ones_b[:],
                            zz[:, j * MM:(j + 1) * MM],
                            start=(t == 0 and j == 0),
                            stop=(t == N_TILES - 1 and j == NZSL - 1))
                        if j == NZSL - 1:
                            mm.then_inc(pe_sem, 1)
                else:
                    # chase the quarter stream of the last tile
                    hs = NSL // 2
                    tensor.wait_ge(qsems[0], 16)
                    for i in range(hs):
                        nc.tensor.matmul(
                            psA[:, :], ones_b[:],
                            xt[:, i * MM:(i + 1) * MM],
                            start=False, stop=False)
                    tensor.wait_ge(qsems[1], 16)
                    for i in range(hs):
                        nc.tensor.matmul(
                            psB[:, :], ones_b[:],
                            xt[:, HALF + i * MM:HALF + (i + 1) * MM],
                            start=False, stop=False)
                    tensor.wait_ge(z_sem, N_TILES)
                    for j in range(NZSL // 2):
                        nc.tensor.matmul(
                            psC[:, :], ones_b[:],
                            zz[:, j * MM:(j + 1) * MM],
                            start=False, stop=False)
                    tensor.wait_ge(qsems[2], 16)
                    for i in range(hs, NSL):
                        nc.tensor.matmul(
                            psA[:, :], ones_b[:],
                            xt[:, i * MM:(i + 1) * MM],
                            start=False, stop=(i == NSL - 1))
                    tensor.wait_ge(qsems[3], 16)
                    for i in range(hs, NSL):
                        nc.tensor.matmul(
                            psB[:, :], ones_b[:],
                            xt[:, HALF + i * MM:HALF + (i + 1) * MM],
                            start=False, stop=(i == NSL - 1))
                    tensor.wait_ge(z_sem, N_TILES + 1)
                    for j in range(NZSL // 2, NZSL):
                        mm = nc.tensor.matmul(
                            psC[:, :], ones_b[:],
                            zz[:, j * MM:(j + 1) * MM],
                            start=False, stop=(j == NZSL - 1))
                        if j == NZSL - 1:
                            mm.then_inc(pe_sem, 1)

        @block.sync
        def _(sync):
            # final [1,48] f32 store: HWDGE, one descriptor, no spray
            sync.wait_ge(dve_sem, 2)
            sync.dma_start(out_d[:, :], res2[:]).then_inc(out_sem, 16)
            if DEBUG_DUMP:
                sync.dma_start(ps_d[:, :], psraw[:]).then_inc(out_sem, 16)
                sync.dma_start(z_d[:, :], zslots[(N_TILES - 1) % NZ][:]
                               ).then_inc(out_sem, 16)
                sync.wait_ge(out_sem, 48)
            else:
                sync.wait_ge(out_sem, 16)

    return nc


def _get_nc():
    if "nc" not in _CACHE:
        _CACHE["nc"] = _build_nc()
    return _CACHE["nc"]


def _pack_core(pred_c, gt_c):
    """[ROWS_PER_CORE, C] f32 0/1 pair -> [N_TILES, P, FREE] bf16 bits.

    bf16(0.0/1.0) == top 16 bits of the f32 pattern, so packing is a
    strided uint16 copy - no float conversion.
    """
    import ml_dtypes
    x = np.empty((N_TILES, P, FREE), dtype=np.uint16)
    # little-endian: high half of each f32 is the second uint16
    x[:, :, 0:HALF] = np.ascontiguousarray(pred_c).reshape(
        N_TILES, P, HALF).view(np.uint16)[..., 1::2]
    x[:, :, HALF:FREE] = np.ascontiguousarray(gt_c).reshape(
        N_TILES, P, HALF).view(np.uint16)[..., 1::2]
    return x.view(ml_dtypes.bfloat16)


def kernel(pred, gt, **run_kwargs):
    global LAST_RUN
    from concourse.bass_utils import run_bass_kernel_spmd

    pred = np.asarray(pred, dtype=np.float32)
    gt = np.asarray(gt, dtype=np.float32)
    assert pred.shape == (N_ROWS, C) and gt.shape == (N_ROWS, C)

    in_maps = []
    for i in range(N_CORES):
        sl = slice(i * ROWS_PER_CORE, (i + 1) * ROWS_PER_CORE)
        in_maps.append({"x": _pack_core(pred[sl], gt[sl])})

    nc = _get_nc()
    br = run_bass_kernel_spmd(nc, in_maps, core_ids=list(range(N_CORES)),
                              **run_kwargs)
    LAST_RUN = br

    partials = np.stack([r["out"].reshape(3 * C) for r in br.results])
    totals = partials.astype(np.float64).sum(axis=0)  # exact integers
    pred_sum = totals[0:C].astype(np.float32)
    gt_sum = totals[C:2 * C].astype(np.float32)
    intersection = totals[2 * C:3 * C].astype(np.float32)

    recalls = (intersection + EPS) / (gt_sum + EPS)
    precisions = (intersection + EPS) / (pred_sum + EPS)
    return (precisions, recalls, intersection, gt_sum, pred_sum)


# revision 14
# speedup vs baseline: 1.7418x; 1.0121x over previous
"""Trainium2 Bass kernel (raw Bass): per-class precision/recall sums.

Computes, for pred/gt 0-1 indicator tensors of shape [N, C]:
    intersection = sum_n pred*gt   [C]
    pred_sum     = sum_n pred      [C]
    gt_sum       = sum_n gt        [C]
    precisions   = (intersection + EPS) / (pred_sum + EPS)
    recalls      = (intersection + EPS) / (gt_sum + EPS)

Sharding: rows split across 8 NeuronCores. The host packs each core's
chunk as bf16 (exact for 0/1 indicators - truncating the f32 top half)
into x[16, 128, 8192]: tile t, partition p holds 256 consecutive rows;
free layout = [pred (q256 c16) | gt (q256 c16)]. bf16 on the wire halves
HBM traffic vs f32: 32 MiB/core, ~79 us at the 16x27GB/s DMA-engine
roofline.

Device pipeline per core:
  - gpsimd SWDGE streams 16 tiles xt[128, 8192] bf16 into 8 rotating
    SBUF slots (no dtype cast - host already packed bf16). Last tile
    split into 4 quarter-DMAs so compute can chase the stream.
  - DVE: z = pred_half * gt_half per tile (one [128,4096] bf16 mul).
  - PE: ones[128,1]^T @ 512-col slices; pred slices accumulate
    psA[1,512], gt slices psB[1,512], z slices psC[1,512]. Stationary
    ones never changes; 24 matmuls/tile, ~420 PE instructions total
    (small enough to avoid mid-stream iram refills, which made DMA
    engine 64 a straggler in the f32/gram version).
  - Epilogue: DVE strided reduces psA/psB/psC -> res[1,48]; sync-engine
    HWDGE writes res to HBM as a single descriptor.
Each core emits [1, 3*C] = [pred_sum, gt_sum, intersection]; the host
sums partials (exact integers in f64) and applies the epsilon math.
"""

from contextlib import ExitStack

import numpy as np

N_CORES = 8
N_ROWS, C = 4194304, 16
ROWS_PER_CORE = N_ROWS // N_CORES  # 524288
EPS = np.float32(1e-6)

P = 128
N_TILES = 16
Q = ROWS_PER_CORE // (N_TILES * P)  # 256 rows per (tile, partition)
HALF = Q * C                        # 4096
FREE = 2 * HALF                     # 8192
N_SLOTS = 8
NZ = 3
MM = 512                            # moving free per matmul
NSL = HALF // MM                    # 8 slices per half
NZSL = HALF // MM                   # 8 slices of z

_CACHE = {}
LAST_RUN = None  # BassKernelResults of the most recent run (for test harness)
DEBUG_DUMP = False  # add raw psum + z-tile debug outputs


def _build_nc():
    import concourse.bass as bass
    import concourse.mybir as mybir

    f32 = mybir.dt.float32
    bf16 = mybir.dt.bfloat16

    nc = bass.Bass()
    x_d = nc.dram_tensor("x", [N_TILES, P, FREE], bf16, kind="ExternalInput")
    out_d = nc.dram_tensor("out", [1, 3 * C], f32, kind="ExternalOutput")
    if DEBUG_DUMP:
        ps_d = nc.dram_tensor("psdump", [1, 3 * MM], f32, kind="ExternalOutput")
        z_d = nc.dram_tensor("zdump", [P, HALF], bf16, kind="ExternalOutput")
    x_t = x_d[:, :, :]

    ctx = ExitStack()
    with ctx:
        ones_b = ctx.enter_context(nc.sbuf_tensor("ones_b", [P, 1], bf16))
        res = ctx.enter_context(nc.sbuf_tensor("res", [1, 3 * C], f32))
        res2 = ctx.enter_context(nc.sbuf_tensor("res2", [1, 3 * C], f32))
        if DEBUG_DUMP:
            psraw = ctx.enter_context(nc.sbuf_tensor("psraw", [1, 3 * MM], f32))
        slots = [
            ctx.enter_context(nc.sbuf_tensor(f"xt{s}", [P, FREE], bf16))
            for s in range(N_SLOTS)
        ]
        zslots = [
            ctx.enter_context(nc.sbuf_tensor(f"z{s}", [P, HALF], bf16))
            for s in range(NZ)
        ]

        psA = ctx.enter_context(nc.psum_tensor([1, MM], f32))
        psB = ctx.enter_context(nc.psum_tensor([1, MM], f32))
        psC = ctx.enter_context(nc.psum_tensor([1, MM], f32))
        psD = ctx.enter_context(nc.psum_tensor([1, 1], f32))

        slot_sems = [
            ctx.enter_context(nc.semaphore(name=f"slot{s}"))
            for s in range(N_SLOTS)
        ]
        qsems = [
            ctx.enter_context(nc.semaphore(name=f"q{k}"))
            for k in range(4)
        ]
        z_sem = ctx.enter_context(nc.semaphore(name="zs"))
        pe_sem = ctx.enter_context(nc.semaphore(name="pe"))
        dve_sem = ctx.enter_context(nc.semaphore(name="dve"))
        out_sem = ctx.enter_context(nc.semaphore(name="outd"))
        block = ctx.enter_context(nc.Block())

        # last tile quarter ranges, issue order: pred_h0, gt_h0, pred_h1,
        # gt_h1 so DVE can mul half 0 while halves 1 stream in.
        QTR = [(0, HALF // 2), (HALF, HALF + HALF // 2),
               (HALF // 2, HALF), (HALF + HALF // 2, FREE)]

        @block.gpsimd
        def _(gpsimd):
            for t in range(N_TILES):
                s = t % N_SLOTS
                if t >= N_SLOTS:
                    # PE finished all matmuls of the slot's previous tile,
                    # which also implies DVE's mul of it completed.
                    gpsimd.wait_ge(pe_sem, t - N_SLOTS + 1)
                if t < N_TILES - 1:
                    gpsimd.dma_start(slots[s][:], x_t[t]).then_inc(
                        slot_sems[s], 16)
                else:
                    for k, (lo, hi) in enumerate(QTR):
                        gpsimd.dma_start(
                            slots[s][:, lo:hi], x_t[t][:, lo:hi],
                        ).then_inc(qsems[k], 16)

        @block.vector
        def _(vector):
            # inc must ride ON the writing instruction: a trailing nop's
            # inc can fire while the previous op's writes are in flight.
            vector.memset(ones_b[:], 1.0).then_inc(dve_sem, 1)
            for t in range(N_TILES - 1):
                s = t % N_SLOTS
                if t >= NZ:
                    vector.wait_ge(pe_sem, t - NZ + 1)
                vector.wait_ge(slot_sems[s], 16 * (t // N_SLOTS + 1))
                vector.tensor_mul(
                    zslots[t % NZ][:],
                    slots[s][:, 0:HALF],
                    slots[s][:, HALF:FREE],
                ).then_inc(z_sem, 1)
            # last tile: two half-muls chasing the quarter DMAs
            t = N_TILES - 1
            s = t % N_SLOTS
            zz = zslots[t % NZ]
            vector.wait_ge(pe_sem, t - NZ + 1)
            h = HALF // 2
            vector.wait_ge(qsems[1], 16)
            vector.tensor_mul(
                zz[:, 0:h], slots[s][:, 0:h], slots[s][:, HALF:HALF + h],
            ).then_inc(z_sem, 1)
            vector.wait_ge(qsems[3], 16)
            vector.tensor_mul(
                zz[:, h:HALF], slots[s][:, h:HALF], slots[s][:, HALF + h:FREE],
            ).then_inc(z_sem, 1)
            # epilogue: decode psum accumulators once PE's settle fence fires
            vector.wait_ge(pe_sem, N_TILES + 1)
            vector.tensor_reduce(
                res[0:1, 0:C],
                psA[:, :].rearrange("p (q c) -> p c q", c=C),
                axis=mybir.AxisListType.X, op=mybir.AluOpType.add)
            vector.tensor_reduce(
                res[0:1, C:2 * C],
                psB[:, :].rearrange("p (q c) -> p c q", c=C),
                axis=mybir.AxisListType.X, op=mybir.AluOpType.add)
            vector.tensor_reduce(
                res[0:1, 2 * C:3 * C],
                psC[:, :].rearrange("p (q c) -> p c q", c=C),
                axis=mybir.AxisListType.X, op=mybir.AluOpType.add)
            if DEBUG_DUMP:
                vector.tensor_copy(psraw[0:1, 0:MM], psA[:, :])
                vector.tensor_copy(psraw[0:1, MM:2 * MM], psB[:, :])
                vector.tensor_copy(psraw[0:1, 2 * MM:3 * MM], psC[:, :])
            # read-back layer: the copy can't read res before the reduces'
            # writes land (in-order same-engine RAW), and its own inc rides
            # on the instruction that writes res2 - the DMA reads res2.
            vector.tensor_copy(res2[0:1, :], res[0:1, :]).then_inc(dve_sem, 1)

        @block.tensor
        def _(tensor):
            tensor.wait_ge(dve_sem, 1)  # ones_b ready
            n_mm = N_TILES * NSL
            n_zmm = N_TILES * NZSL
            for t in range(N_TILES):
                s = t % N_SLOTS
                xt = slots[s]
                zz = zslots[t % NZ]
                if t < N_TILES - 1:
                    tensor.wait_ge(slot_sems[s], 16 * (t // N_SLOTS + 1))
                    for i in range(NSL):
                        nc.tensor.matmul(
                            psA[:, :], ones_b[:],
                            xt[:, i * MM:(i + 1) * MM],
                            start=(t == 0 and i == 0),
                            stop=(t == N_TILES - 1 and i == NSL - 1))
                    for i in range(NSL):
                        nc.tensor.matmul(
                            psB[:, :], ones_b[:],
                            xt[:, HALF + i * MM:HALF + (i + 1) * MM],
                            start=(t == 0 and i == 0),
                            stop=(t == N_TILES - 1 and i == NSL - 1))
                    tensor.wait_ge(z_sem, t + 1)
                    for j in range(NZSL):
                        mm = nc.tensor.matmul(
                            psC[:, :], ones_b[:],
                            zz[:, j * MM:(j + 1) * MM],
                            start=(t == 0 and j == 0),
                            stop=(t == N_TILES - 1 and j == NZSL - 1))
                        if j == NZSL - 1:
                            mm.then_inc(pe_sem, 1)
                else:
                    # chase the quarter stream of the last tile
                    hs = NSL // 2
                    tensor.wait_ge(qsems[0], 16)
                    for i in range(hs):
                        nc.tensor.matmul(
                            psA[:, :], ones_b[:],
                            xt[:, i * MM:(i + 1) * MM],
                            start=False, stop=False)
                    tensor.wait_ge(qsems[1], 16)
                    for i in range(hs):
                        nc.tensor.matmul(
                            psB[:, :], ones_b[:],
                            xt[:, HALF + i * MM:HALF + (i + 1) * MM],
                            start=False, stop=False)
                    tensor.wait_ge(z_sem, N_TILES)
                    for j in range(NZSL // 2):
                        nc.tensor.matmul(
                            psC[:, :], ones_b[:],
                            zz[:, j * MM:(j + 1) * MM],
                            start=False, stop=False)
                    tensor.wait_ge(qsems[2], 16)
                    for i in range(hs, NSL):
                        nc.tensor.matmul(
                            psA[:, :], ones_b[:],
                            xt[:, i * MM:(i + 1) * MM],
                            start=False, stop=(i == NSL - 1))
                    tensor.wait_ge(qsems[3], 16)
                    for i in range(hs, NSL):
                        nc.tensor.matmul(
                            psB[:, :], ones_b[:],
                            xt[:, HALF + i * MM:HALF + (i + 1) * MM],
                            start=False, stop=(i == NSL - 1))
                    tensor.wait_ge(z_sem, N_TILES + 1)
                    for j in range(NZSL // 2, NZSL):
                        mm = nc.tensor.matmul(
                            psC[:, :], ones_b[:],
                            zz[:, j * MM:(j + 1) * MM],
                            start=False, stop=(j == NZSL - 1))
                        if j == NZSL - 1:
                            mm.then_inc(pe_sem, 1)
            # settle fence: the PE array retires in order, so when this
            # dummy's writes land every prior PSUM accumulation has landed.
            # The epilogue readers wait for its inc, not the last real mm's.
            nc.tensor.matmul(psD[:, :], ones_b[:], ones_b[:],
                             start=True, stop=True).then_inc(pe_sem, 1)

        @block.sync
        def _(sync):
            # final [1,48] f32 store: HWDGE, one descriptor, no spray
            sync.wait_ge(dve_sem, 2)
            sync.dma_start(out_d[:, :], res2[:]).then_inc(out_sem, 16)
            if DEBUG_DUMP:
                sync.dma_start(ps_d[:, :], psraw[:]).then_inc(out_sem, 16)
                sync.dma_start(z_d[:, :], zslots[(N_TILES - 1) % NZ][:]
                               ).then_inc(out_sem, 16)
                sync.wait_ge(out_sem, 48)
            else:
                sync.wait_ge(out_sem, 16)

    return nc


def _get_nc():
    if "nc" not in _CACHE:
        _CACHE["nc"] = _build_nc()
    return _CACHE["nc"]


def _pack_core(pred_c, gt_c):
    """[ROWS_PER_CORE, C] f32 0/1 pair -> [N_TILES, P, FREE] bf16 bits.

    bf16(0.0/1.0) == top 16 bits of the f32 pattern, so packing is a
    strided uint16 copy - no float conversion.
    """
    import ml_dtypes
    x = np.empty((N_TILES, P, FREE), dtype=np.uint16)
    # little-endian: high half of each f32 is the second uint16
    x[:, :, 0:HALF] = np.ascontiguousarray(pred_c).reshape(
        N_TILES, P, HALF).view(np.uint16)[..., 1::2]
    x[:, :, HALF:FREE] = np.ascontiguousarray(gt_c).reshape(
        N_TILES, P, HALF).view(np.uint16)[..., 1::2]
    return x.view(ml_dtypes.bfloat16)


def kernel(pred, gt, **run_kwargs):
    global LAST_RUN
    from concourse.bass_utils import run_bass_kernel_spmd

    pred = np.asarray(pred, dtype=np.float32)
    gt = np.asarray(gt, dtype=np.float32)
    assert pred.shape == (N_ROWS, C) and gt.shape == (N_ROWS, C)

    in_maps = []
    for i in range(N_CORES):
        sl = slice(i * ROWS_PER_CORE, (i + 1) * ROWS_PER_CORE)
        in_maps.append({"x": _pack_core(pred[sl], gt[sl])})

    nc = _get_nc()
    br = run_bass_kernel_spmd(nc, in_maps, core_ids=list(range(N_CORES)),
                              **run_kwargs)
    LAST_RUN = br

    partials = np.stack([r["out"].reshape(3 * C) for r in br.results])
    totals = partials.astype(np.float64).sum(axis=0)  # exact integers
    pred_sum = totals[0:C].astype(np.float32)
    gt_sum = totals[C:2 * C].astype(np.float32)
    intersection = totals[2 * C:3 * C].astype(np.float32)

    recalls = (intersection + EPS) / (gt_sum + EPS)
    precisions = (intersection + EPS) / (pred_sum + EPS)
    return (precisions, recalls, intersection, gt_sum, pred_sum)


# revision 30
# speedup vs baseline: 1.7684x; 1.0152x over previous
"""Trainium2 Bass kernel (raw Bass): per-class precision/recall sums.

Computes, for pred/gt 0-1 indicator tensors of shape [N, C]:
    intersection = sum_n pred*gt   [C]
    pred_sum     = sum_n pred      [C]
    gt_sum       = sum_n gt        [C]
    precisions   = (intersection + EPS) / (pred_sum + EPS)
    recalls      = (intersection + EPS) / (gt_sum + EPS)

Sharding: rows split across 8 NeuronCores. The host packs each core's
chunk as bf16 (exact for 0/1 indicators - truncating the f32 top half)
into x[16, 128, 8192]: tile t, partition p holds 256 consecutive rows;
free layout = [pred (q256 c16) | gt (q256 c16)]. bf16 on the wire halves
HBM traffic vs f32: 32 MiB/core, ~80 us at the 16x27GB/s DMA-engine
roofline.

Device pipeline per core:
  - sync-engine HWDGE streams 16 tiles xt[128, 8192] bf16 into 8
    rotating SBUF slots. Last tile split into 8 eighth-DMAs so compute
    chases the stream.
  - DVE per tile: z = pred_half * gt_half ([128,4096] bf16 mul), then
    two contiguous pairwise folds (halves add: 4096->2048->1024). The
    class lanes stay aligned (cell u -> class u%16) and values <= 4 are
    exact in bf16. This cuts PE's z matmuls from 8 to 2 per tile,
    keeping PE (18 mm/tile ~ 3.9us) under the DMA rate (~4.9us/tile).
  - PE: ones^T @ 512-col slices; pred slices accumulate psA[1,512], gt
    slices psB[1,512], folded-z slices psC[1,512]. The z matmuls for
    tile t run in iteration t+1 so PE never waits on DVE.
  - Settle fence: dummy matmul whose completion implies all prior PSUM
    writes landed (sem incs can fire before the write pipeline drains).
  - Epilogue: DVE strided reduces psA/psB/psC -> res[1,48], copy to
    res2 (read-back layer), sync HWDGE stores res2 as one descriptor.
Each core emits [1, 3*C] = [pred_sum, gt_sum, intersection]; the host
sums partials (exact integers in f64) and applies the epsilon math.
"""

from contextlib import ExitStack

import numpy as np

N_CORES = 8
N_ROWS, C = 4194304, 16
ROWS_PER_CORE = N_ROWS // N_CORES  # 524288
EPS = np.float32(1e-6)

P = 128
N_TILES = 16
Q = ROWS_PER_CORE // (N_TILES * P)  # 256 rows per (tile, partition)
HALF = Q * C                        # 4096
FREE = 2 * HALF                     # 8192
N_SLOTS = 8
NZ = 3
MM = 512                            # moving free per matmul
NSL = HALF // MM                    # 8 slices per half
ZF = HALF // 4                      # 1024 cols of twice-folded z
NZSL = ZF // MM                     # 2 z slices per tile

_CACHE = {}
LAST_RUN = None  # BassKernelResults of the most recent run (for test harness)


def _build_nc():
    import concourse.bass as bass
    import concourse.mybir as mybir

    f32 = mybir.dt.float32
    bf16 = mybir.dt.bfloat16

    nc = bass.Bass()
    x_d = nc.dram_tensor("x", [N_TILES, P, FREE], bf16, kind="ExternalInput")
    out_d = nc.dram_tensor("out", [1, 3 * C], f32, kind="ExternalOutput")
    x_t = x_d[:, :, :]

    ctx = ExitStack()
    with ctx:
        ones_b = ctx.enter_context(nc.sbuf_tensor("ones_b", [P, 1], bf16))
        res = ctx.enter_context(nc.sbuf_tensor("res", [1, 3 * C], f32))
        res2 = ctx.enter_context(nc.sbuf_tensor("res2", [1, 3 * C], f32))
        slots = [
            ctx.enter_context(nc.sbuf_tensor(f"xt{s}", [P, FREE], bf16))
            for s in range(N_SLOTS)
        ]
        zslots = [
            ctx.enter_context(nc.sbuf_tensor(f"z{s}", [P, HALF], bf16))
            for s in range(NZ)
        ]
        zf1s = [
            ctx.enter_context(nc.sbuf_tensor(f"zf1_{s}", [P, HALF // 2], bf16))
            for s in range(NZ)
        ]
        zf2s = [
            ctx.enter_context(nc.sbuf_tensor(f"zf2_{s}", [P, ZF], bf16))
            for s in range(NZ)
        ]

        psA = ctx.enter_context(nc.psum_tensor([1, MM], f32))
        psB = ctx.enter_context(nc.psum_tensor([1, MM], f32))
        psC = ctx.enter_context(nc.psum_tensor([1, MM], f32))
        psD = ctx.enter_context(nc.psum_tensor([1, 1], f32))

        slot_sems = [
            ctx.enter_context(nc.semaphore(name=f"slot{s}"))
            for s in range(N_SLOTS)
        ]
        qsems = [
            ctx.enter_context(nc.semaphore(name=f"q{k}"))
            for k in range(8)
        ]
        z_sem = ctx.enter_context(nc.semaphore(name="zs"))
        dself = ctx.enter_context(nc.semaphore(name="dself"))
        pe_sem = ctx.enter_context(nc.semaphore(name="pe"))
        dve_sem = ctx.enter_context(nc.semaphore(name="dve"))
        out_sem = ctx.enter_context(nc.semaphore(name="outd"))
        block = ctx.enter_context(nc.Block())

        # last tile eighths, issue order pred_q/gt_q interleaved so each
        # (pred, gt) pair completes as early as possible for DVE's muls.
        E8 = HALF // 4  # 1024 cols per eighth
        EIGHTHS = []
        for j in range(4):
            EIGHTHS.append((j * E8, (j + 1) * E8))                    # pred
            EIGHTHS.append((HALF + j * E8, HALF + (j + 1) * E8))      # gt
        LAST = N_TILES - 1

        def qwait(engine, k):
            engine.wait_ge(qsems[k], 16)

        @block.sync
        def _(sync):
            for t in range(N_TILES):
                s = t % N_SLOTS
                if t >= N_SLOTS:
                    # PE retired iteration t-8 (its sum reads of the slot);
                    # z_sem at t-7 means DVE's fold2(t-8), hence its mul of
                    # the slot, completed. Both readers must be done.
                    sync.wait_ge(pe_sem, t - N_SLOTS + 1)
                    sync.wait_ge(z_sem, t - N_SLOTS + 1)
                if t < LAST:
                    sync.dma_start(slots[s][:], x_t[t]).then_inc(
                        slot_sems[s], 16)
                else:
                    for k, (lo, hi) in enumerate(EIGHTHS):
                        sync.dma_start(
                            slots[s][:, lo:hi], x_t[t][:, lo:hi],
                        ).then_inc(qsems[k], 16)
            # final [1,48] f32 store: HWDGE, one descriptor, no spray
            sync.wait_ge(dve_sem, 2)
            sync.dma_start(out_d[:, :], res2[:]).then_inc(out_sem, 16)
            sync.wait_ge(out_sem, 16)

        @block.vector
        def _(vector):
            # incs ride ON the writing instruction: a trailing nop's inc
            # can fire while the previous op's writes are in flight.
            vector.memset(ones_b[:], 1.0).then_inc(dve_sem, 1)
            H2, H4 = HALF // 2, HALF // 4
            # DVE does NOT interlock same-engine read-after-write: each
            # producer incs dself and its same-engine consumer waits.
            nv = 0
            for t in range(N_TILES - 1):
                s = t % N_SLOTS
                zz, z1, z2 = zslots[t % NZ], zf1s[t % NZ], zf2s[t % NZ]
                vector.wait_ge(slot_sems[s], 16 * (t // N_SLOTS + 1))
                vector.tensor_mul(
                    zz[:], slots[s][:, 0:HALF], slots[s][:, HALF:FREE],
                ).then_inc(dself, 1)
                nv += 1
                vector.wait_ge(dself, nv)
                vector.tensor_add(
                    z1[:], zz[:, 0:H2], zz[:, H2:HALF]).then_inc(dself, 1)
                nv += 1
                if t >= 2:
                    # PE's iteration t-2 retired -> zf2[t%3] free
                    vector.wait_ge(pe_sem, t - 1)
                vector.wait_ge(dself, nv)
                vector.tensor_add(
                    z2[:], z1[:, 0:H4], z1[:, H4:H2]).then_inc(z_sem, 1)
            # last tile: per-quarter mul+fold chasing the eighth-DMAs;
            # PE consumes zf1 quarters directly (values <= 2, one mm each)
            t = LAST
            s = t % N_SLOTS
            zz, z1 = zslots[t % NZ], zf1s[t % NZ]
            vector.wait_ge(pe_sem, t - 1)
            qe = HALF // 4
            for j in range(4):
                qwait(vector, 2 * j + 1)
                vector.tensor_mul(
                    zz[:, j * qe:(j + 1) * qe],
                    slots[s][:, j * qe:(j + 1) * qe],
                    slots[s][:, HALF + j * qe:HALF + (j + 1) * qe],
                ).then_inc(dself, 1)
                nv += 1
                vector.wait_ge(dself, nv)
                vector.tensor_add(
                    z1[:, j * MM:(j + 1) * MM],
                    zz[:, j * qe:j * qe + MM],
                    zz[:, j * qe + MM:(j + 1) * qe]).then_inc(z_sem, 1)
            # epilogue: decode psums once PE's settle fence fires
            vector.wait_ge(pe_sem, N_TILES + 1)
            vector.tensor_reduce(
                res[0:1, 0:C],
                psA[:, :].rearrange("p (q c) -> p c q", c=C),
                axis=mybir.AxisListType.X, op=mybir.AluOpType.add
            ).then_inc(dself, 1)
            vector.tensor_reduce(
                res[0:1, C:2 * C],
                psB[:, :].rearrange("p (q c) -> p c q", c=C),
                axis=mybir.AxisListType.X, op=mybir.AluOpType.add
            ).then_inc(dself, 1)
            vector.tensor_reduce(
                res[0:1, 2 * C:3 * C],
                psC[:, :].rearrange("p (q c) -> p c q", c=C),
                axis=mybir.AxisListType.X, op=mybir.AluOpType.add
            ).then_inc(dself, 1)
            nv += 3
            # read-back layer: gated on the reduces' own completion sems,
            # and its inc rides on the instruction that writes res2 - the
            # out-DMA reads res2.
            vector.wait_ge(dself, nv)
            vector.tensor_copy(res2[0:1, :], res[0:1, :]).then_inc(dve_sem, 1)

        @block.tensor
        def _(tensor):
            tensor.wait_ge(dve_sem, 1)  # ones ready
            for t in range(N_TILES - 1):
                s = t % N_SLOTS
                xt = slots[s]
                tensor.wait_ge(slot_sems[s], 16 * (t // N_SLOTS + 1))
                for i in range(NSL):
                    nc.tensor.matmul(
                        psA[:, :], ones_b[:],
                        xt[:, i * MM:(i + 1) * MM],
                        start=(t == 0 and i == 0), stop=False)
                last = None
                for i in range(NSL):
                    last = nc.tensor.matmul(
                        psB[:, :], ones_b[:],
                        xt[:, HALF + i * MM:HALF + (i + 1) * MM],
                        start=(t == 0 and i == 0), stop=False)
                if t > 0:
                    # deferred: folded z of tile t-1 (long ready, no stall)
                    zp = zf2s[(t - 1) % NZ]
                    tensor.wait_ge(z_sem, t)
                    for j in range(NZSL):
                        last = nc.tensor.matmul(
                            psC[:, :], ones_b[:],
                            zp[:, j * MM:(j + 1) * MM],
                            start=(t == 1 and j == 0), stop=False)
                last.then_inc(pe_sem, 1)
            # last tile: z(14) first, then chase eighths + z quarters
            t = LAST
            xt = slots[t % N_SLOTS]
            zp = zf2s[(t - 1) % NZ]
            z1 = zf1s[t % NZ]
            tensor.wait_ge(z_sem, t)
            for j in range(NZSL):
                nc.tensor.matmul(
                    psC[:, :], ones_b[:], zp[:, j * MM:(j + 1) * MM],
                    start=False, stop=False)
            for j in range(4):
                qwait(tensor, 2 * j)
                nc.tensor.matmul(
                    psA[:, :], ones_b[:],
                    xt[:, 2 * j * MM:(2 * j + 1) * MM],
                    start=False, stop=False)
                nc.tensor.matmul(
                    psA[:, :], ones_b[:],
                    xt[:, (2 * j + 1) * MM:(2 * j + 2) * MM],
                    start=False, stop=(j == 3))
                qwait(tensor, 2 * j + 1)
                nc.tensor.matmul(
                    psB[:, :], ones_b[:],
                    xt[:, HALF + 2 * j * MM:HALF + (2 * j + 1) * MM],
                    start=False, stop=False)
                nc.tensor.matmul(
                    psB[:, :], ones_b[:],
                    xt[:, HALF + (2 * j + 1) * MM:HALF + (2 * j + 2) * MM],
                    start=False, stop=(j == 3))
                tensor.wait_ge(z_sem, t + 1 + j)
                mm = nc.tensor.matmul(
                    psC[:, :], ones_b[:], z1[:, j * MM:(j + 1) * MM],
                    start=False, stop=(j == 3))
                if j == 3:
                    mm.then_inc(pe_sem, 1)
            # settle fence: the PE array retires in order, so when this
            # dummy lands every prior PSUM accumulation has landed.
            nc.tensor.matmul(psD[:, :], ones_b[:], ones_b[:],
                             start=True, stop=True).then_inc(pe_sem, 1)

    return nc


def _get_nc():
    if "nc" not in _CACHE:
        _CACHE["nc"] = _build_nc()
    return _CACHE["nc"]


def _pack_core(pred_c, gt_c):
    """[ROWS_PER_CORE, C] f32 0/1 pair -> [N_TILES, P, FREE] bf16 bits.

    bf16(0.0/1.0) == top 16 bits of the f32 pattern, so packing is a
    strided uint16 copy - no float conversion.
    """
    import ml_dtypes
    x = np.empty((N_TILES, P, FREE), dtype=np.uint16)
    # little-endian: high half of each f32 is the second uint16
    x[:, :, 0:HALF] = np.ascontiguousarray(pred_c).reshape(
        N_TILES, P, HALF).view(np.uint16)[..., 1::2]
    x[:, :, HALF:FREE] = np.ascontiguousarray(gt_c).reshape(
        N_TILES, P, HALF).view(np.uint16)[..., 1::2]
    return x.view(ml_dtypes.bfloat16)


def kernel(pred, gt, **run_kwargs):
    global LAST_RUN
    from concourse.bass_utils import run_bass_kernel_spmd

    pred = np.asarray(pred, dtype=np.float32)
    gt = np.asarray(gt, dtype=np.float32)
    assert pred.shape == (N_ROWS, C) and gt.shape == (N_ROWS, C)

    in_maps = []
    for i in range(N_CORES):
        sl = slice(i * ROWS_PER_CORE, (i + 1) * ROWS_PER_CORE)
        in_maps.append({"x": _pack_core(pred[sl], gt[sl])})

    nc = _get_nc()
    br = run_bass_kernel_spmd(nc, in_maps, core_ids=list(range(N_CORES)),
                              **run_kwargs)
    LAST_RUN = br

    partials = np.stack([r["out"].reshape(3 * C) for r in br.results])
    totals = partials.astype(np.float64).sum(axis=0)  # exact integers
    pred_sum = totals[0:C].astype(np.float32)
    gt_sum = totals[C:2 * C].astype(np.float32)
    intersection = totals[2 * C:3 * C].astype(np.float32)

    recalls = (intersection + EPS) / (gt_sum + EPS)
    precisions = (intersection + EPS) / (pred_sum + EPS)
    return (precisions, recalls, intersection, gt_sum, pred_sum)


# revision 31
# speedup vs baseline: 1.8155x; 1.0266x over previous
"""Trainium2 Bass kernel (raw Bass): per-class precision/recall sums.

Computes, for pred/gt 0-1 indicator tensors of shape [N, C]:
    intersection = sum_n pred*gt   [C]
    pred_sum     = sum_n pred      [C]
    gt_sum       = sum_n gt        [C]
    precisions   = (intersection + EPS) / (pred_sum + EPS)
    recalls      = (intersection + EPS) / (gt_sum + EPS)

Sharding: rows split across 8 NeuronCores. The host packs each core's
chunk as bf16 (exact for 0/1 indicators - truncating the f32 top half)
into x[16, 128, 8192]: tile t, partition p holds 256 consecutive rows;
free layout = [pred (q256 c16) | gt (q256 c16)]. bf16 on the wire halves
HBM traffic vs f32: 32 MiB/core, ~80 us at the 16x27GB/s DMA-engine
roofline.

Device pipeline per core:
  - sync-engine HWDGE streams 16 tiles xt[128, 8192] bf16 into 8
    rotating SBUF slots. Last tile split into 8 eighth-DMAs so compute
    chases the stream.
  - DVE per tile: z = pred_half * gt_half ([128,4096] bf16 mul), then
    two contiguous pairwise folds (halves add: 4096->2048->1024). The
    class lanes stay aligned (cell u -> class u%16) and values <= 4 are
    exact in bf16. This cuts PE's z matmuls from 8 to 2 per tile,
    keeping PE (18 mm/tile ~ 3.9us) under the DMA rate (~4.9us/tile).
  - PE: ones^T @ 512-col slices; pred slices accumulate psA[1,512], gt
    slices psB[1,512], folded-z slices psC[1,512]. The z matmuls for
    tile t run in iteration t+1 so PE never waits on DVE.
  - Settle fence: dummy matmul whose completion implies all prior PSUM
    writes landed (sem incs can fire before the write pipeline drains).
  - Epilogue: DVE strided reduces psA/psB/psC -> res[1,48], copy to
    res2 (read-back layer), sync HWDGE stores res2 as one descriptor.
Each core emits [1, 3*C] = [pred_sum, gt_sum, intersection]; the host
sums partials (exact integers in f64) and applies the epsilon math.
"""

from contextlib import ExitStack

import numpy as np

N_CORES = 8
N_ROWS, C = 4194304, 16
ROWS_PER_CORE = N_ROWS // N_CORES  # 524288
EPS = np.float32(1e-6)

P = 128
N_TILES = 16
Q = ROWS_PER_CORE // (N_TILES * P)  # 256 rows per (tile, partition)
HALF = Q * C                        # 4096
FREE = 2 * HALF                     # 8192
N_SLOTS = 8
NZ = 3
MM = 512                            # moving free per matmul
NSL = HALF // MM                    # 8 slices per half
ZF = HALF // 4                      # 1024 cols of twice-folded z
NZSL = ZF // MM                     # 2 z slices per tile

_CACHE = {}
LAST_RUN = None  # BassKernelResults of the most recent run (for test harness)


def _build_nc():
    import concourse.bass as bass
    import concourse.mybir as mybir

    f32 = mybir.dt.float32
    bf16 = mybir.dt.bfloat16

    nc = bass.Bass()
    x_d = nc.dram_tensor("x", [N_TILES, P, FREE], bf16, kind="ExternalInput")
    out_d = nc.dram_tensor("out", [1, 3 * C], f32, kind="ExternalOutput")
    x_t = x_d[:, :, :]

    ctx = ExitStack()
    with ctx:
        ones_b = ctx.enter_context(nc.sbuf_tensor("ones_b", [P, 1], bf16))
        res = ctx.enter_context(nc.sbuf_tensor("res", [1, 3 * C], f32))
        res2 = ctx.enter_context(nc.sbuf_tensor("res2", [1, 3 * C], f32))
        slots = [
            ctx.enter_context(nc.sbuf_tensor(f"xt{s}", [P, FREE], bf16))
            for s in range(N_SLOTS)
        ]
        zslots = [
            ctx.enter_context(nc.sbuf_tensor(f"z{s}", [P, HALF], bf16))
            for s in range(NZ)
        ]
        zf1s = [
            ctx.enter_context(nc.sbuf_tensor(f"zf1_{s}", [P, HALF // 2], bf16))
            for s in range(NZ)
        ]

        psA = ctx.enter_context(nc.psum_tensor([1, MM], f32))
        psB = ctx.enter_context(nc.psum_tensor([1, MM], f32))
        psC = ctx.enter_context(nc.psum_tensor([1, MM], f32))
        psD = ctx.enter_context(nc.psum_tensor([1, 1], f32))

        slot_sems = [
            ctx.enter_context(nc.semaphore(name=f"slot{s}"))
            for s in range(N_SLOTS)
        ]
        qsems = [
            ctx.enter_context(nc.semaphore(name=f"q{k}"))
            for k in range(8)
        ]
        z_sem = ctx.enter_context(nc.semaphore(name="zs"))
        dself = ctx.enter_context(nc.semaphore(name="dself"))
        pe_sem = ctx.enter_context(nc.semaphore(name="pe"))
        dve_sem = ctx.enter_context(nc.semaphore(name="dve"))
        out_sem = ctx.enter_context(nc.semaphore(name="outd"))
        block = ctx.enter_context(nc.Block())

        # last tile eighths, issue order pred_q/gt_q interleaved so each
        # (pred, gt) pair completes as early as possible for DVE's muls.
        E8 = HALF // 4  # 1024 cols per eighth
        EIGHTHS = []
        for j in range(4):
            EIGHTHS.append((j * E8, (j + 1) * E8))                    # pred
            EIGHTHS.append((HALF + j * E8, HALF + (j + 1) * E8))      # gt
        LAST = N_TILES - 1

        def qwait(engine, k):
            engine.wait_ge(qsems[k], 16)

        @block.sync
        def _(sync):
            for t in range(N_TILES):
                s = t % N_SLOTS
                if t >= N_SLOTS:
                    # PE retired iteration t-8 (its sum reads of the slot);
                    # z_sem at t-7 means DVE's fold2(t-8), hence its mul of
                    # the slot, completed. Both readers must be done.
                    sync.wait_ge(pe_sem, t - N_SLOTS + 1)
                    sync.wait_ge(z_sem, t - N_SLOTS + 1)
                if t < LAST:
                    sync.dma_start(slots[s][:], x_t[t]).then_inc(
                        slot_sems[s], 16)
                else:
                    for k, (lo, hi) in enumerate(EIGHTHS):
                        sync.dma_start(
                            slots[s][:, lo:hi], x_t[t][:, lo:hi],
                        ).then_inc(qsems[k], 16)
            # final [1,48] f32 store: HWDGE, one descriptor, no spray
            sync.wait_ge(dve_sem, 2)
            sync.dma_start(out_d[:, :], res2[:]).then_inc(out_sem, 16)
            sync.wait_ge(out_sem, 16)

        @block.vector
        def _(vector):
            # incs ride ON the writing instruction: a trailing nop's inc
            # can fire while the previous op's writes are in flight.
            vector.memset(ones_b[:], 1.0).then_inc(dve_sem, 1)
            H2, H4 = HALF // 2, HALF // 4
            # DVE does NOT interlock same-engine read-after-write: each
            # producer incs dself and its same-engine consumer waits.
            nv = 0
            for t in range(N_TILES - 1):
                s = t % N_SLOTS
                zz, z1 = zslots[t % NZ], zf1s[t % NZ]
                vector.wait_ge(slot_sems[s], 16 * (t // N_SLOTS + 1))
                vector.tensor_mul(
                    zz[:], slots[s][:, 0:HALF], slots[s][:, HALF:FREE],
                ).then_inc(dself, 1)
                nv += 1
                if t >= 2:
                    # PE's iteration t-2 retired -> zf1[t%3] free
                    vector.wait_ge(pe_sem, t - 1)
                vector.wait_ge(dself, nv)
                vector.tensor_add(
                    z1[:], zz[:, 0:H2], zz[:, H2:HALF]).then_inc(z_sem, 1)
            # last tile: per-quarter mul+fold chasing the eighth-DMAs;
            # PE consumes zf1 quarters directly (values <= 2, one mm each)
            t = LAST
            s = t % N_SLOTS
            zz, z1 = zslots[t % NZ], zf1s[t % NZ]
            vector.wait_ge(pe_sem, t - 1)
            qe = HALF // 4
            for j in range(4):
                qwait(vector, 2 * j + 1)
                vector.tensor_mul(
                    zz[:, j * qe:(j + 1) * qe],
                    slots[s][:, j * qe:(j + 1) * qe],
                    slots[s][:, HALF + j * qe:HALF + (j + 1) * qe],
                ).then_inc(dself, 1)
                nv += 1
                vector.wait_ge(dself, nv)
                vector.tensor_add(
                    z1[:, j * MM:(j + 1) * MM],
                    zz[:, j * qe:j * qe + MM],
                    zz[:, j * qe + MM:(j + 1) * qe]).then_inc(z_sem, 1)
            # epilogue: psA/psB stopped at the eighth sums (fenceAB ->
            # pe_sem 16); psC stops after the quarter z-mms (fence -> 18).
            vector.wait_ge(pe_sem, N_TILES)
            vector.tensor_reduce(
                res[0:1, 0:C],
                psA[:, :].rearrange("p (q c) -> p c q", c=C),
                axis=mybir.AxisListType.X, op=mybir.AluOpType.add
            ).then_inc(dself, 1)
            vector.tensor_reduce(
                res[0:1, C:2 * C],
                psB[:, :].rearrange("p (q c) -> p c q", c=C),
                axis=mybir.AxisListType.X, op=mybir.AluOpType.add
            ).then_inc(dself, 1)
            vector.wait_ge(pe_sem, N_TILES + 2)
            vector.tensor_reduce(
                res[0:1, 2 * C:3 * C],
                psC[:, :].rearrange("p (q c) -> p c q", c=C),
                axis=mybir.AxisListType.X, op=mybir.AluOpType.add
            ).then_inc(dself, 1)
            nv += 3
            # read-back layer: gated on the reduces' own completion sems,
            # and its inc rides on the instruction that writes res2 - the
            # out-DMA reads res2.
            vector.wait_ge(dself, nv)
            vector.tensor_copy(res2[0:1, :], res[0:1, :]).then_inc(dve_sem, 1)

        @block.tensor
        def _(tensor):
            tensor.wait_ge(dve_sem, 1)  # ones ready
            for t in range(N_TILES - 1):
                s = t % N_SLOTS
                xt = slots[s]
                tensor.wait_ge(slot_sems[s], 16 * (t // N_SLOTS + 1))
                for i in range(NSL):
                    nc.tensor.matmul(
                        psA[:, :], ones_b[:],
                        xt[:, i * MM:(i + 1) * MM],
                        start=(t == 0 and i == 0), stop=False)
                last = None
                for i in range(NSL):
                    last = nc.tensor.matmul(
                        psB[:, :], ones_b[:],
                        xt[:, HALF + i * MM:HALF + (i + 1) * MM],
                        start=(t == 0 and i == 0), stop=False)
                if t > 0:
                    # deferred: folded z of tile t-1 (long ready, no stall)
                    zp = zf1s[(t - 1) % NZ]
                    tensor.wait_ge(z_sem, t)
                    for j in range(4):
                        last = nc.tensor.matmul(
                            psC[:, :], ones_b[:],
                            zp[:, j * MM:(j + 1) * MM],
                            start=(t == 1 and j == 0), stop=False)
                last.then_inc(pe_sem, 1)
            # last tile: z(14) first, then chase eighths + z quarters
            t = LAST
            xt = slots[t % N_SLOTS]
            zp = zf1s[(t - 1) % NZ]
            z1 = zf1s[t % NZ]
            tensor.wait_ge(z_sem, t)
            for j in range(4):
                nc.tensor.matmul(
                    psC[:, :], ones_b[:], zp[:, j * MM:(j + 1) * MM],
                    start=False, stop=False)
            for j in range(4):
                qwait(tensor, 2 * j)
                nc.tensor.matmul(
                    psA[:, :], ones_b[:],
                    xt[:, 2 * j * MM:(2 * j + 1) * MM],
                    start=False, stop=False)
                nc.tensor.matmul(
                    psA[:, :], ones_b[:],
                    xt[:, (2 * j + 1) * MM:(2 * j + 2) * MM],
                    start=False, stop=(j == 3))
                qwait(tensor, 2 * j + 1)
                nc.tensor.matmul(
                    psB[:, :], ones_b[:],
                    xt[:, HALF + 2 * j * MM:HALF + (2 * j + 1) * MM],
                    start=False, stop=False)
                nc.tensor.matmul(
                    psB[:, :], ones_b[:],
                    xt[:, HALF + (2 * j + 1) * MM:HALF + (2 * j + 2) * MM],
                    start=False, stop=(j == 3))
            # fenceAB: psA/psB are final -> DVE may reduce them (pe_sem 16)
            nc.tensor.matmul(psD[:, :], ones_b[:], ones_b[:],
                             start=True, stop=False).then_inc(pe_sem, 1)
            for j in range(4):
                tensor.wait_ge(z_sem, t + 1 + j)
                mm = nc.tensor.matmul(
                    psC[:, :], ones_b[:], z1[:, j * MM:(j + 1) * MM],
                    start=False, stop=(j == 3))
                if j == 3:
                    mm.then_inc(pe_sem, 1)
            # settle fence: the PE array retires in order, so when this
            # dummy lands every prior PSUM accumulation has landed.
            nc.tensor.matmul(psD[:, :], ones_b[:], ones_b[:],
                             start=False, stop=True).then_inc(pe_sem, 1)

    return nc


def _get_nc():
    if "nc" not in _CACHE:
        _CACHE["nc"] = _build_nc()
    return _CACHE["nc"]


def _pack_core(pred_c, gt_c):
    """[ROWS_PER_CORE, C] f32 0/1 pair -> [N_TILES, P, FREE] bf16 bits.

    bf16(0.0/1.0) == top 16 bits of the f32 pattern, so packing is a
    strided uint16 copy - no float conversion.
    """
    import ml_dtypes
    x = np.empty((N_TILES, P, FREE), dtype=np.uint16)
    # little-endian: high half of each f32 is the second uint16
    x[:, :, 0:HALF] = np.ascontiguousarray(pred_c).reshape(
        N_TILES, P, HALF).view(np.uint16)[..., 1::2]
    x[:, :, HALF:FREE] = np.ascontiguousarray(gt_c).reshape(
        N_TILES, P, HALF).view(np.uint16)[..., 1::2]
    return x.view(ml_dtypes.bfloat16)


def kernel(pred, gt, **run_kwargs):
    global LAST_RUN
    from concourse.bass_utils import run_bass_kernel_spmd

    pred = np.asarray(pred, dtype=np.float32)
    gt = np.asarray(gt, dtype=np.float32)
    assert pred.shape == (N_ROWS, C) and gt.shape == (N_ROWS, C)

    in_maps = []
    for i in range(N_CORES):
        sl = slice(i * ROWS_PER_CORE, (i + 1) * ROWS_PER_CORE)
        in_maps.append({"x": _pack_core(pred[sl], gt[sl])})

    nc = _get_nc()
    br = run_bass_kernel_spmd(nc, in_maps, core_ids=list(range(N_CORES)),
                              **run_kwargs)
    LAST_RUN = br

    partials = np.stack([r["out"].reshape(3 * C) for r in br.results])
    totals = partials.astype(np.float64).sum(axis=0)  # exact integers
    pred_sum = totals[0:C].astype(np.float32)
    gt_sum = totals[C:2 * C].astype(np.float32)
    intersection = totals[2 * C:3 * C].astype(np.float32)

    recalls = (intersection + EPS) / (gt_sum + EPS)
    precisions = (intersection + EPS) / (pred_sum + EPS)
    return (precisions, recalls, intersection, gt_sum, pred_sum)
